# revision 9
# baseline (speedup 1.0000x reference)
"""Trainium2 Bass kernel for nn_BaseContextAwareModel (4-layer GCN + mean-pool + conv1d head).

Strategy (per the graph-id sharding hint):
- Each of the 1920 22-node frame-graphs is independent -> shard 240 graphs/core
  across 8 NeuronCores (== 2 batch items per core, since 120 graphs = one clip).
- On host: build the dense normalized adjacency Ahat (G,22,22) from
  edge_index/edge_attr (GCN norm: D^-1/2 (A+I) D^-1/2), then pack 5 graphs per
  128-partition tile as a 110x110 block-diagonal stationary (transposed).
- On device, per 110-node chunk and GCN layer l:
    mm1: M1t = H_c^T AhatT_c     (lhsT = H_c, rhs = AhatT block)   -> PSUM (C_l, 110)
    mm2: H'  = M1t^T W_l (+ b_l) (lhsT = M1t,  rhs = W_l)          -> PSUM (110, C_out)
  with relu fused into the PSUM->SBUF eviction. All matmuls bf16 with f32 PSUM.
- The PSUM->SBUF traffic (m1 copies + relu evictions) is the bottleneck, so it
  is load-balanced across the Activation, DVE and Pool(GPSIMD) engines by a
  greedy finish-time scheduler; m1 supertiles of 8 chunks (2 PSUM banks) keep
  the instruction count low. PE emission is software-pipelined (mm1 of group
  g+1 before mm2 of group g) so the PE never waits on an eviction engine.
- Inputs ship in 6 large DMAs (vs per-tile DMAs): HWDGE fixed cost and SP
  sequencer serialization otherwise dominate the startup.
- PE p-state is warmed with a chain of dummy matmuls at t=0 so real matmuls
  run at full clock; the ACT sigmoid table is preloaded at t=0 by a warmup op.
- Head: mean-pool via 0/1 matmul into per-clip PSUM accumulators (1/22 and the
  uniform eval-BN scale folded into conv weights on host), conv1d(k=3) as
  shifted-lhsT matmuls, sigmoid on ACT, (s-.5)^2 and the capsule reduction on
  DVE (hiding the act-table switch to Sqrt), one combined Sqrt, one output
  DMA. Clip 0's head chain overlaps the second half of layer 3.
"""

import os
from contextlib import ExitStack

import numpy as np

import concourse.bass as bass
import concourse.bacc as bacc
import concourse.tile as tile
from concourse import mybir
from concourse.bass_utils import run_bass_kernel_spmd

# ---- problem constants (hardcoded; kernel.py must be self-contained) ----
BS, T, P, G = 16, 120, 22, 1920
NCORES = 8
GPC = G // NCORES          # 240 graphs per core
CPG = 5                    # graphs per 128-partition chunk
CH = CPG * P               # 110 nodes per chunk
NCHUNK = GPC // CPG        # 48 chunks per core
BPC = BS // NCORES         # 2 batch items (clips) per core
KPB = T // CPG             # 24 chunks per clip
C_IN = 14
CHS = [16, 32, 64, 152]
DIMS = [C_IN] + CHS
NCLS, DIM_CAP = 17, 16
C_CONV = DIM_CAP * NCLS    # 272
BN_EPS = 1e-3

MS = 8                     # chunks per m1 supertile (2 PSUM banks, 4 per bank)
H_CPB = {16: 8, 32: 16, 64: 8, 152: 3}   # chunks per 1-bank h supertile

F32 = mybir.dt.float32
BF16 = mybir.dt.bfloat16
NPBF16 = np.dtype(mybir.dt.np(BF16))

TRACE = os.environ.get("KTRACE", "0") == "1"
LAST = None  # last BassKernelResults, for test harness introspection
LAST_NC = None  # last built bass.Bass module, for cost-model simulation

# const-pack column layout (bf16):
# pack1: W2 [0:16,0:32] | W3 [0:32,32:96] | W4 [0:64,96:248] | poolm [0:110,248:253]
P1_W2, P1_W3, P1_W4, P1_PM = 0, 32, 96, 248
P1_COLS = 253
# pack2: wc1 [0:128,0:816] | wc2 [0:24,816:1632] | ones [0:1,1632:1760]
P2_WC1, P2_WC2, P2_ONES = 0, 816, 1632
P2_COLS = 1760

# bd DMA split (chunks per slice)
BD_SPLIT = [8, 16, 24]
BD_BASE = [0, 8, 24, 48]


def _host_prep(x, edge_index, edge_attr, conv_w, W1, bn_gamma, bn_beta):
    """Dense normalized adjacency + per-core packed operands."""
    src = np.asarray(edge_index[0], np.int64)
    dst = np.asarray(edge_index[1], np.int64)
    w = np.asarray(edge_attr[:, 4], np.float32)

    A = np.zeros((G, P, P), np.float32)
    np.add.at(A, (dst // P, dst % P, src % P), w)
    deg = A.sum(axis=2) + 1.0                      # + self-loop weight 1
    dinv = 1.0 / np.sqrt(deg)                      # deg >= 1 always
    Ahat = dinv[:, :, None] * A * dinv[:, None, :]
    ii = np.arange(P)
    Ahat[:, ii, ii] += dinv * dinv                 # self loop: dinv[d]^2
    AhatT = np.ascontiguousarray(Ahat.transpose(0, 2, 1))  # [g, s, d]

    # block-diag pack: (NCORES, CH, NCHUNK*CH); rows = source node in chunk,
    # cols = chunk*CH + dest node in chunk
    bd = np.zeros((NCORES, CH, NCHUNK * CH), np.float32)
    bdv = bd.reshape(NCORES, CH, NCHUNK, CH)
    Ar = AhatT.reshape(NCORES, NCHUNK, CPG, P, P)
    for j in range(CPG):
        bdv[:, j * P:(j + 1) * P, :, j * P:(j + 1) * P] = \
            Ar[:, :, j].transpose(0, 2, 1, 3)

    # layer-1 W folded on host: ship XW1 = x @ W1, packed (8, 110, 48, 16)
    xw = np.asarray(x, np.float32) @ np.asarray(W1, np.float32)
    xr = xw.reshape(NCORES, NCHUNK, CH, CHS[0])
    xp = np.ascontiguousarray(xr.transpose(0, 2, 1, 3))

    # conv weights: (co, ci, k) -> (ci, k, co), with the 1/22 mean-pool factor.
    # BatchNorm(eval) with uniform gamma and zero beta folds into the conv
    # weights as a scalar (the conv bias is scaled on the host side too).
    gamma = np.asarray(bn_gamma, np.float32)
    beta = np.asarray(bn_beta, np.float32)
    bn_uniform = bool(np.allclose(gamma, gamma[0]) and not np.any(beta))
    cscale = float(gamma[0] / np.sqrt(1.0 + BN_EPS)) if bn_uniform else 1.0
    wct = np.asarray(conv_w, np.float32).transpose(1, 2, 0) * (cscale / float(P))
    return bd.astype(NPBF16), xp.astype(NPBF16), np.ascontiguousarray(wct), \
        bn_uniform, cscale


class EvictSched:
    """Greedy finish-time balancing of PSUM->SBUF copies/activations across
    the ACT, DVE and Pool engines (engine-busy cost model)."""

    def __init__(self, nc):
        self.nc = nc
        # Pool/GPSIMD cannot access PSUM (BIR verifier), so evictions can
        # only run on the Activation and DVE engines.
        self.est = {"act": 0.0, "dve": 0.0}

    def _cost(self, eng, cols):
        if eng == "act":
            return cols * 0.833 + 185.0
        if eng == "dve":
            return cols * 1.042 + 125.0
        return cols * 1.389 + 95.0

    def _pick(self, cols):
        eng = min(self.est, key=lambda e: self.est[e] + self._cost(e, cols))
        self.est[eng] += self._cost(eng, cols)
        return eng

    def note(self, eng, cols):
        """Account externally emitted work (pinned head ops)."""
        self.est[eng] += self._cost(eng, cols)

    def copy(self, dst, src, cols, pin=None):
        eng = pin or self._pick(cols)
        if pin:
            self.note(eng, cols)
        nc = self.nc
        if eng == "act":
            nc.scalar.activation(dst, src, mybir.ActivationFunctionType.Copy)
        elif eng == "dve":
            nc.vector.tensor_copy(dst, src)
        else:
            nc.gpsimd.tensor_copy(dst, src)
        return eng

    def relu(self, dst, src, cols, pin=None):
        eng = pin or self._pick(cols)
        if pin:
            self.note(eng, cols)
        nc = self.nc
        if eng == "act":
            nc.scalar.activation(dst, src, mybir.ActivationFunctionType.Relu)
        elif eng == "dve":
            nc.vector.tensor_scalar_max(dst, src, 0.0)
        else:
            nc.gpsimd.tensor_scalar_max(dst, src, 0.0)
        return eng


def _build(nonzero_b, nonzero_convb, bn_uniform, cscale):
    """Build the SPMD Bass program (identical on all 8 cores)."""
    nc = bacc.Bacc()
    AF = mybir.ActivationFunctionType

    d_bd = [nc.declare_dram_parameter(f"bd{i}", [CH, n * CH], BF16, isOutput=False)
            for i, n in enumerate(BD_SPLIT)]
    d_x = nc.declare_dram_parameter("xp", [CH, NCHUNK, CHS[0]], BF16, isOutput=False)
    d_p1 = nc.declare_dram_parameter("pack1", [128, P1_COLS], BF16, isOutput=False)
    d_p2 = nc.declare_dram_parameter("pack2", [128, P2_COLS], BF16, isOutput=False)
    d_b = [nc.declare_dram_parameter(f"b{l}", [1, DIMS[l + 1]], BF16, isOutput=False)
           if nonzero_b[l] else None for l in range(4)]
    d_convb = (nc.declare_dram_parameter("convb", [1, C_CONV], BF16, isOutput=False)
               if nonzero_convb else None)
    if not bn_uniform:
        d_bnscale = nc.declare_dram_parameter("bnscale", [T, 1], F32, isOutput=False)
        d_bnshift = nc.declare_dram_parameter("bnshift", [T, 1], F32, isOutput=False)
    d_out = nc.declare_dram_parameter("out", [BPC * T, NCLS], F32, isOutput=True)

    with tile.TileContext(nc) as tc, ExitStack() as ctx:
        const = ctx.enter_context(tc.tile_pool(name="const", bufs=1))
        state = ctx.enter_context(tc.tile_pool(name="state", bufs=1))
        m1p = ctx.enter_context(tc.tile_pool(name="m1sb", bufs=3))
        psA = ctx.enter_context(tc.tile_pool(name="psA", bufs=2, space="PSUM"))
        psB = ctx.enter_context(tc.tile_pool(name="psB", bufs=2, space="PSUM"))
        psC = ctx.enter_context(tc.tile_pool(name="psC", bufs=1, space="PSUM"))
        head = ctx.enter_context(tc.tile_pool(name="head", bufs=1))

        sched = EvictSched(nc)

        # ---- warmups (no DMA dependencies) ----
        t_negh = const.tile([128, 1], F32)
        nc.gpsimd.memset(t_negh, -0.5)
        t_dummy = const.tile([1, 512], BF16)
        nc.vector.memset(t_dummy, 0.0)
        # ACT warmup: triggers the sigmoid act-table load at t~0 (the only
        # other table needed is Sqrt, switched once at the tail).
        t_warm = head.tile([128, 1], F32, tag="warm")
        nc.scalar.activation(t_warm, t_negh, AF.Sigmoid)

        # ---- input DMAs (SP engine, ordered by first use) ----
        t_bd = []
        for i, n in enumerate(BD_SPLIT):
            tb = const.tile([CH, n * CH], BF16, tag=f"bd{i}")
            t_bd.append(tb)
        nc.sync.dma_start(out=t_bd[0], in_=d_bd[0][:])
        t_x = const.tile([CH, NCHUNK, CHS[0]], BF16)
        nc.sync.dma_start(out=t_x, in_=d_x[:])
        t_p1 = const.tile([128, P1_COLS], BF16)
        nc.sync.dma_start(out=t_p1, in_=d_p1[:])
        nc.sync.dma_start(out=t_bd[1], in_=d_bd[1][:])
        nc.sync.dma_start(out=t_bd[2], in_=d_bd[2][:])
        t_p2 = const.tile([128, P2_COLS], BF16)
        nc.sync.dma_start(out=t_p2, in_=d_p2[:])
        t_b = []
        for l in range(4):
            if d_b[l] is not None:
                tb = const.tile([1, DIMS[l + 1]], BF16, tag=f"bias{l}")
                nc.sync.dma_start(out=tb, in_=d_b[l][:])
                t_b.append(tb)
            else:
                t_b.append(None)
        t_convb = None
        if d_convb is not None:
            t_convb = const.tile([1, C_CONV], BF16)
            nc.sync.dma_start(out=t_convb, in_=d_convb[:])
        t_bnscale = t_bnshift = None
        if not bn_uniform:
            t_bnscale = const.tile([T, 1], F32)
            nc.sync.dma_start(out=t_bnscale, in_=d_bnscale[:])
            t_bnshift = const.tile([T, 1], F32)
            nc.sync.dma_start(out=t_bnshift, in_=d_bnshift[:])

        def ahat_chunk(k):
            i = 0 if k < BD_BASE[1] else (1 if k < BD_BASE[2] else 2)
            return t_bd[i][:, (k - BD_BASE[i]) * CH:(k - BD_BASE[i] + 1) * CH]

        t_w = {1: t_p1[0:16, P1_W2:P1_W2 + 32],
               2: t_p1[0:32, P1_W3:P1_W3 + 64],
               3: t_p1[0:64, P1_W4:P1_W4 + 152]}
        t_poolm = t_p1[0:CH, P1_PM:P1_PM + CPG]
        t_ones = t_p2[0:1, P2_ONES:P2_ONES + 128]
        t_wc1 = t_p2[0:128, P2_WC1:P2_WC1 + 816].rearrange("p (k c) -> p k c", k=3)
        t_wc2 = t_p2[0:CHS[3] - 128, P2_WC2:P2_WC2 + 816].rearrange(
            "p (k c) -> p k c", k=3)

        # ---- PE p-state warmup: chained dummy matmuls, partitions 96-97 of
        # the first m1-tag PSUM tile (disjoint from m1 copies' cin<=64 rows).
        ps_warm = psA.tile([128, 1024], F32, tag="m1")
        for _ in range(8):
            nc.tensor.matmul(ps_warm[64:65, 0:512], lhsT=t_dummy[:, 0:1],
                             rhs=t_dummy[:, :], start=True, stop=True)

        # ---- head state (per-clip accumulators, persist in psC) ----
        ps_pool1 = psC.tile([128, 512], F32, tag="pool1")
        ps_pool2 = psC.tile([128, 512], F32, tag="pool2")
        t_pt1 = [head.tile([128, T + 2], BF16, tag=f"pt1_{b}", name=f"pt1_{b}")
                 for b in range(BPC)]
        t_pt2 = [head.tile([CHS[3] - 128, T + 2], BF16, tag=f"pt2_{b}",
                           name=f"pt2_{b}") for b in range(BPC)]
        t_s = [head.tile([T, C_CONV], F32, tag=f"s{b}", name=f"s{b}")
               for b in range(BPC)]
        t_q = [head.tile([T, C_CONV], F32, tag=f"q{b}", name=f"q{b}")
               for b in range(BPC)]
        t_ssum = head.tile([T, BPC * NCLS], F32, tag="ssum")
        t_y = head.tile([T, BPC * NCLS], F32, tag="y")
        for b in range(BPC):
            for pt in (t_pt1[b], t_pt2[b]):
                nc.gpsimd.memset(pt[:, 0:1], 0.0)
                nc.gpsimd.memset(pt[:, T + 1:T + 2], 0.0)

        def head_after_htile(h3, h_base, h_fill):
            """Mean-pool matmuls for a fresh L3 h-tile; full clip -> conv +
            sigmoid + capsule chain (clip 0 overlaps the rest of L3)."""
            for j in range(h_fill):
                k = h_base + j
                kk = k % KPB
                nc.tensor.matmul(ps_pool1[:, kk * CPG:(kk + 1) * CPG],
                                 lhsT=h3[:, k, :128], rhs=t_poolm[:],
                                 start=True, stop=True)
                nc.tensor.matmul(ps_pool2[:CHS[3] - 128, kk * CPG:(kk + 1) * CPG],
                                 lhsT=h3[:, k, 128:], rhs=t_poolm[:],
                                 start=True, stop=True)
            if (h_base + h_fill) % KPB != 0:
                return
            b = (h_base + h_fill) // KPB - 1
            sched.copy(t_pt1[b][:, 1:T + 1], ps_pool1[:, :T], T)
            sched.copy(t_pt2[b][:, 1:T + 1], ps_pool2[:CHS[3] - 128, :T], T)
            ps_caps = psB.tile([128, 512], F32, tag="h")
            first = True
            for ci, (pt, twc) in enumerate(((t_pt1[b], t_wc1), (t_pt2[b], t_wc2))):
                for kk in range(3):
                    last = (t_convb is None) and ci == 1 and kk == 2
                    nc.tensor.matmul(ps_caps[:T, :C_CONV], lhsT=pt[:, kk:kk + T],
                                     rhs=twc[:, kk, :], start=first, stop=last,
                                     skip_group_check=True)
                    first = False
            if t_convb is not None:
                nc.tensor.matmul(ps_caps[:T, :C_CONV], lhsT=t_ones[:, :T],
                                 rhs=t_convb[:], start=False, stop=True,
                                 skip_group_check=True)
            if bn_uniform:
                nc.scalar.activation(t_s[b], ps_caps[:T, :C_CONV], AF.Sigmoid)
            else:
                nc.scalar.activation(t_s[b], ps_caps[:T, :C_CONV], AF.Sigmoid,
                                     bias=t_bnshift[:, 0:1],
                                     scale=t_bnscale[:, 0:1])
            sched.note("act", C_CONV)
            nc.vector.tensor_scalar_add(t_q[b], t_s[b], -0.5)
            nc.vector.tensor_mul(t_q[b], t_q[b], t_q[b])
            nc.vector.reduce_sum(
                out=t_ssum[:, b * NCLS:(b + 1) * NCLS],
                in_=t_q[b].rearrange("p (d c) -> p c d", c=NCLS),
                axis=mybir.AxisListType.X)
            sched.note("dve", 3 * C_CONV)

        # ---- L0: W1 folded into xp -> one matmul per chunk; 8-chunk tiles
        # aligned with the bd slice arrival so L1 starts as data lands.
        h0 = state.tile([CH, NCHUNK, CHS[0]], BF16, tag="h0")
        for c0 in range(0, NCHUNK, MS):
            ps_h = psB.tile([128, 512], F32, tag="h")
            for j in range(MS):
                k = c0 + j
                nc.tensor.matmul(
                    ps_h[:CH, j * CHS[0]:(j + 1) * CHS[0]],
                    lhsT=ahat_chunk(k), rhs=t_x[:, k, :],
                    start=True, stop=(t_b[0] is None))
                if t_b[0] is not None:
                    nc.tensor.matmul(
                        ps_h[:CH, j * CHS[0]:(j + 1) * CHS[0]],
                        lhsT=t_ones[:, :CH], rhs=t_b[0][:],
                        start=False, stop=True, skip_group_check=True)
            dst = h0[:, c0:c0 + MS, :].rearrange("p a b -> p (a b)")
            sched.relu(dst, ps_h[:CH, :MS * CHS[0]], MS * CHS[0])
        h_prev = h0

        # ---- L1..L3: software-pipelined mm1/copy/mm2 per 8-chunk group ----
        ngroups = NCHUNK // MS
        for l in range(1, 4):
            cin, cout = DIMS[l], DIMS[l + 1]
            cpb = H_CPB[cout]
            h_next = state.tile([CH, NCHUNK, cout], BF16, tag=f"h{l}")

            m1_sbs = [None] * ngroups
            ps_h = None
            h_fill = 0
            h_base = 0

            def emit_mm2_chunk(k, m1_sb, j):
                nonlocal ps_h, h_fill, h_base
                if ps_h is None:
                    ps_h = psB.tile([128, 512], F32, tag="h")
                    h_fill = 0
                    h_base = k
                nc.tensor.matmul(
                    ps_h[:CH, h_fill * cout:(h_fill + 1) * cout],
                    lhsT=m1_sb[:cin, j * CH:(j + 1) * CH],
                    rhs=t_w[l][:, :],
                    start=True, stop=(t_b[l] is None))
                if t_b[l] is not None:
                    nc.tensor.matmul(
                        ps_h[:CH, h_fill * cout:(h_fill + 1) * cout],
                        lhsT=t_ones[:, :CH], rhs=t_b[l][:],
                        start=False, stop=True, skip_group_check=True)
                h_fill += 1
                if h_fill == cpb or k == NCHUNK - 1:
                    dst = h_next[:, h_base:h_base + h_fill, :].rearrange(
                        "p a b -> p (a b)")
                    sched.relu(dst, ps_h[:CH, :h_fill * cout], h_fill * cout)
                    if l == 3:
                        head_after_htile(h_next, h_base, h_fill)
                    ps_h = None

            for g in range(ngroups + 1):
                if g < ngroups:
                    g0 = g * MS
                    ps_m1 = psA.tile([128, 1024], F32, tag="m1")
                    for j in range(MS):
                        k = g0 + j
                        bank, slot = divmod(j, 4)
                        nc.tensor.matmul(
                            ps_m1[:cin, bank * 512 + slot * CH:
                                  bank * 512 + (slot + 1) * CH],
                            lhsT=h_prev[:, k, :cin], rhs=ahat_chunk(k),
                            start=True, stop=True)
                    m1_sb = m1p.tile([128, MS * CH], BF16, tag="m1sb")
                    # one copy per PSUM bank (cross-bank compute APs are not
                    # HW-legal even though CoreSim accepts them)
                    for bank in range(2):
                        sched.copy(
                            m1_sb[:cin, bank * 4 * CH:(bank + 1) * 4 * CH],
                            ps_m1[:cin, bank * 512:bank * 512 + 4 * CH],
                            4 * CH)
                    m1_sbs[g] = m1_sb
                # mm2 for the PREVIOUS group (software pipeline: PE works on
                # mm1(g) while the copy of group g-1 is in flight)
                if g > 0:
                    pg = g - 1
                    for j in range(MS):
                        emit_mm2_chunk(pg * MS + j, m1_sbs[pg], j)
            h_prev = h_next

        # ---- tail: combined sqrt + one output DMA ----
        nc.scalar.activation(t_y, t_ssum, AF.Sqrt, scale=4.0 / DIM_CAP)
        nc.sync.dma_start(
            out=d_out[:].rearrange("(b t) c -> t b c", b=BPC),
            in_=t_y.rearrange("p (b c) -> p b c", b=BPC))

    return nc


def kernel(x, edge_index, batch, edge_attr, W1, b1, W2, b2, W3, b3, W4, b4,
           conv_w, conv_b, bn_gamma, bn_beta):
    global LAST, LAST_NC
    bd, xp, wct, bn_uniform, cscale = _host_prep(
        x, edge_index, edge_attr, conv_w, W1, bn_gamma, bn_beta)

    bs = [np.asarray(b_, np.float32) for b_ in (b1, b2, b3, b4)]
    nonzero_b = [bool(np.any(b_)) for b_ in bs]
    convb = np.asarray(conv_b, np.float32) * cscale
    nonzero_convb = bool(np.any(convb))

    poolm = np.zeros((CH, CPG), np.float32)
    for j in range(CPG):
        poolm[j * P:(j + 1) * P, j] = 1.0

    pack1 = np.zeros((128, P1_COLS), np.float32)
    pack1[0:16, P1_W2:P1_W2 + 32] = np.asarray(W2, np.float32)
    pack1[0:32, P1_W3:P1_W3 + 64] = np.asarray(W3, np.float32)
    pack1[0:64, P1_W4:P1_W4 + 152] = np.asarray(W4, np.float32)
    pack1[0:CH, P1_PM:P1_PM + CPG] = poolm

    pack2 = np.zeros((128, P2_COLS), np.float32)
    pack2[0:128, P2_WC1:P2_WC1 + 816] = wct[:128].reshape(128, -1)
    pack2[0:CHS[3] - 128, P2_WC2:P2_WC2 + 816] = wct[128:].reshape(CHS[3] - 128, -1)
    pack2[0:1, P2_ONES:P2_ONES + 128] = 1.0

    nc = _build(nonzero_b, nonzero_convb, bn_uniform, cscale)
    if not nc.is_finalized():
        nc.finalize()   # Bacc: runs the wait-splitting/regalloc compile passes
    LAST_NC = nc

    bnscale = (np.asarray(bn_gamma, np.float32) / np.sqrt(1.0 + BN_EPS)).reshape(T, 1)
    bnshift = np.asarray(bn_beta, np.float32).reshape(T, 1)

    in_maps = []
    for c in range(NCORES):
        m = dict(
            xp=np.ascontiguousarray(xp[c]),
            pack1=pack1.astype(NPBF16),
            pack2=pack2.astype(NPBF16),
        )
        for i, n in enumerate(BD_SPLIT):
            m[f"bd{i}"] = np.ascontiguousarray(
                bd[c][:, BD_BASE[i] * CH:(BD_BASE[i] + n) * CH])
        for l in range(4):
            if nonzero_b[l]:
                m[f"b{l}"] = bs[l].reshape(1, -1).astype(NPBF16)
        if nonzero_convb:
            m["convb"] = convb.reshape(1, -1).astype(NPBF16)
        if not bn_uniform:
            m["bnscale"] = bnscale
            m["bnshift"] = bnshift
        in_maps.append(m)

    LAST = run_bass_kernel_spmd(nc, in_maps, core_ids=list(range(NCORES)),
                                trace=TRACE)
    outs = [LAST.results[c]["out"] for c in range(NCORES)]
    return np.concatenate(outs, axis=0).reshape(BS, T, NCLS)


# revision 27
# speedup vs baseline: 1.1316x; 1.1316x over previous
"""Trainium2 Bass kernel for nn_BaseContextAwareModel (4-layer GCN + mean-pool + conv1d head).

Strategy (per the graph-id sharding hint):
- Each of the 1920 22-node frame-graphs is independent -> shard 240 graphs/core
  across 8 NeuronCores (== 2 batch items per core, since 120 graphs = one clip).
- On host: build the dense normalized adjacency Ahat (G,22,22) from
  edge_index/edge_attr (GCN norm: D^-1/2 (A+I) D^-1/2), then pack 5 graphs per
  128-partition tile as a 110x110 block-diagonal stationary (transposed).
- On device, per 110-node chunk and GCN layer l:
    mm1: M1t = H_c^T AhatT_c     (lhsT = H_c, rhs = AhatT block)   -> PSUM (C_l, 110)
    mm2: H'  = M1t^T W_l (+ b_l) (lhsT = M1t,  rhs = W_l)          -> PSUM (110, C_out)
  with relu fused into the PSUM->SBUF eviction. All matmuls bf16 with f32 PSUM.
- mm1 outputs for 2-3 chunks are stacked at PSUM partition bases {0,32,64}
  (PE-legal tile columns), so each PSUM->SBUF m1 copy moves 2-3 chunks per
  110-column slot: 2-3x less eviction free-size. mm2 consumes a whole stacked
  slot with ONE matmul against a block-diagonal W stack, keeping its tile base
  fixed at 0 (switching the PE row-tile base between matmuls crashes the
  runtime). L3 (cin=64) stacks pairs plus unstacked singles so h-tiles stay at
  3 chunks (456 cols, one PSUM bank; cross-bank compute APs are illegal).
- The PSUM->SBUF traffic (m1 copies + relu evictions) is balanced across the
  Activation and DVE engines by a greedy finish-time scheduler (Pool/GPSIMD
  cannot access PSUM; it does SBUF memsets instead).
- Inputs ship in ~9 large DMAs (HWDGE fixed cost and SP sequencer time
  dominate small DMAs); the block-diag adjacency streams in 8-chunk slices so
  compute starts as soon as the first slice lands.
- PE p-state is warmed with a short chain of dummy matmuls at t=0; the ACT
  sigmoid table is preloaded at t=0 by a warmup op (the only other table,
  Sqrt, is switched to once at the tail, hidden behind DVE work).
- Head: mean-pool via 0/1 matmul into a per-clip PSUM accumulator bank (1/22
  and the uniform eval-BN scale folded into conv weights on host), conv1d(k=3)
  as shifted-lhsT matmuls split into t-blocks [0,64) and [64,120) so most of
  the chain runs before the last chunks finish, sigmoid on ACT, (s-.5)^2/4 and
  the capsule reduction on DVE, one combined Sqrt, one output DMA.
"""

import os
from contextlib import ExitStack

import numpy as np

import concourse.bass as bass
import concourse.bacc as bacc
import concourse.tile as tile
from concourse import mybir
from concourse.bass_utils import run_bass_kernel_spmd

# ---- problem constants (hardcoded; kernel.py must be self-contained) ----
BS, T, P, G = 16, 120, 22, 1920
NCORES = 8
GPC = G // NCORES          # 240 graphs per core
CPG = 5                    # graphs per 128-partition chunk
CH = CPG * P               # 110 nodes per chunk
NCHUNK = GPC // CPG        # 48 chunks per core
BPC = BS // NCORES         # 2 batch items (clips) per core
KPB = T // CPG             # 24 chunks per clip
C_IN = 14
CHS = [16, 32, 64, 152]
DIMS = [C_IN] + CHS
NCLS, DIM_CAP = 17, 16
C_CONV = DIM_CAP * NCLS    # 272
BN_EPS = 1e-3

C0P = 32                   # L0 output channels padded 16->32 (device memset)
H_CPB = {32: 12, 64: 6, 152: 3}   # chunks per 1-bank h supertile
# conv/head t-blocks: (t0, tn); second block at partition base 64 (PE-legal)
TBLK = [(0, 64), (64, 56)]

F32 = mybir.dt.float32
BF16 = mybir.dt.bfloat16
NPBF16 = np.dtype(mybir.dt.np(BF16))

TRACE = os.environ.get("KTRACE", "0") == "1"
LAST = None  # last BassKernelResults, for test harness introspection
LAST_NC = None  # last built bass.Bass module, for cost-model simulation

# const-pack column layout (bf16):
# pack1: W2blk (96,96) | W3blk (96,192) | W4blk (128,304) | poolm (110,5)
P1_W2B, P1_W3B, P1_W4B, P1_PM = 0, 96, 288, 592
P1_COLS = 597
# pack2: wc1 [0:128,0:816] | wc2 [0:24,816:1632] | ones [0:1,1632:1760]
P2_WC1, P2_WC2, P2_ONES = 0, 816, 1632
P2_COLS = 1760

# bd DMA split (chunks per slice)
BD_SPLIT = [8, 8, 8, 8, 8, 8]
BD_BASE = list(np.cumsum([0] + BD_SPLIT))


def _host_prep(x, edge_index, edge_attr, conv_w, W1, bn_gamma, bn_beta):
    """Dense normalized adjacency + per-core packed operands."""
    src = np.asarray(edge_index[0], np.int64)
    dst = np.asarray(edge_index[1], np.int64)
    w = np.asarray(edge_attr[:, 4], np.float32)

    A = np.zeros((G, P, P), np.float32)
    np.add.at(A, (dst // P, dst % P, src % P), w)
    deg = A.sum(axis=2) + 1.0                      # + self-loop weight 1
    dinv = 1.0 / np.sqrt(deg)                      # deg >= 1 always
    Ahat = dinv[:, :, None] * A * dinv[:, None, :]
    ii = np.arange(P)
    Ahat[:, ii, ii] += dinv * dinv                 # self loop: dinv[d]^2
    AhatT = np.ascontiguousarray(Ahat.transpose(0, 2, 1))  # [g, s, d]

    # block-diag pack: (NCORES, CH, NCHUNK*CH); rows = source node in chunk,
    # cols = chunk*CH + dest node in chunk
    bd = np.zeros((NCORES, CH, NCHUNK * CH), np.float32)
    bdv = bd.reshape(NCORES, CH, NCHUNK, CH)
    Ar = AhatT.reshape(NCORES, NCHUNK, CPG, P, P)
    for j in range(CPG):
        bdv[:, j * P:(j + 1) * P, :, j * P:(j + 1) * P] = \
            Ar[:, :, j].transpose(0, 2, 1, 3)

    # layer-1 W folded on host: ship XW1 = x @ W1, packed (8, 110, 48, 16)
    xw = np.asarray(x, np.float32) @ np.asarray(W1, np.float32)
    xr = xw.reshape(NCORES, NCHUNK, CH, CHS[0])
    xp = np.ascontiguousarray(xr.transpose(0, 2, 1, 3))

    # conv weights: (co, ci, k) -> (ci, k, co), with the 1/22 mean-pool factor.
    # BatchNorm(eval) with uniform gamma and zero beta folds into the conv
    # weights as a scalar (the conv bias is scaled on the host side too).
    gamma = np.asarray(bn_gamma, np.float32)
    beta = np.asarray(bn_beta, np.float32)
    bn_uniform = bool(np.allclose(gamma, gamma[0]) and not np.any(beta))
    cscale = float(gamma[0] / np.sqrt(1.0 + BN_EPS)) if bn_uniform else 1.0
    wct = np.asarray(conv_w, np.float32).transpose(1, 2, 0) * (cscale / float(P))
    return bd.astype(NPBF16), xp.astype(NPBF16), np.ascontiguousarray(wct), \
        bn_uniform, cscale


class EvictSched:
    """Greedy finish-time balancing of PSUM->SBUF copies/activations across
    the ACT and DVE engines (Pool/GPSIMD cannot access PSUM)."""

    def __init__(self, nc):
        self.nc = nc
        self.est = {"act": 0.0, "dve": 0.0}

    def _cost(self, eng, cols):
        if eng == "act":
            return cols * 0.833 + 185.0
        return cols * 1.042 + 125.0

    def _pick(self, cols):
        eng = min(self.est, key=lambda e: self.est[e] + self._cost(e, cols))
        self.est[eng] += self._cost(eng, cols)
        return eng

    def note(self, eng, cols):
        self.est[eng] += self._cost(eng, cols)

    def copy(self, dst, src, cols, pin=None):
        eng = pin or self._pick(cols)
        if pin:
            self.note(eng, cols)
        nc = self.nc
        if eng == "act":
            nc.scalar.activation(dst, src, mybir.ActivationFunctionType.Copy)
        else:
            nc.vector.tensor_copy(dst, src)
        return eng

    def relu(self, dst, src, cols, pin=None):
        eng = pin or self._pick(cols)
        if pin:
            self.note(eng, cols)
        nc = self.nc
        if eng == "act":
            nc.scalar.activation(dst, src, mybir.ActivationFunctionType.Relu)
        else:
            nc.vector.tensor_scalar_max(dst, src, 0.0)
        return eng


def _build(nonzero_b, nonzero_convb, bn_uniform, cscale):
    """Build the SPMD Bass program (identical on all 8 cores)."""
    nc = bacc.Bacc()
    AF = mybir.ActivationFunctionType

    d_bd = [nc.declare_dram_parameter(f"bd{i}", [CH, n * CH], BF16, isOutput=False)
            for i, n in enumerate(BD_SPLIT)]
    d_x = nc.declare_dram_parameter("xp", [CH, NCHUNK, CHS[0]], BF16, isOutput=False)
    d_p1 = nc.declare_dram_parameter("pack1", [128, P1_COLS], BF16, isOutput=False)
    d_p2 = nc.declare_dram_parameter("pack2", [128, P2_COLS], BF16, isOutput=False)
    d_b = [nc.declare_dram_parameter(f"b{l}", [1, DIMS[l + 1]], BF16, isOutput=False)
           if nonzero_b[l] else None for l in range(4)]
    d_convb = (nc.declare_dram_parameter("convb", [1, C_CONV], BF16, isOutput=False)
               if nonzero_convb else None)
    if not bn_uniform:
        d_bnscale = nc.declare_dram_parameter("bnscale", [T, 1], F32, isOutput=False)
        d_bnshift = nc.declare_dram_parameter("bnshift", [T, 1], F32, isOutput=False)
    d_out = nc.declare_dram_parameter("out", [BPC * T, NCLS], F32, isOutput=True)

    with tile.TileContext(nc) as tc, ExitStack() as ctx:
        const = ctx.enter_context(tc.tile_pool(name="const", bufs=1))
        state = ctx.enter_context(tc.tile_pool(name="state", bufs=1))
        m1p = ctx.enter_context(tc.tile_pool(name="m1sb", bufs=6))
        psA = ctx.enter_context(tc.tile_pool(name="psA", bufs=4, space="PSUM"))
        psB = ctx.enter_context(tc.tile_pool(name="psB", bufs=3, space="PSUM"))
        psC = ctx.enter_context(tc.tile_pool(name="psC", bufs=1, space="PSUM"))
        head = ctx.enter_context(tc.tile_pool(name="head", bufs=1))

        sched = EvictSched(nc)

        # ---- warmups (no DMA dependencies) ----
        t_negh = const.tile([128, 1], F32)
        nc.gpsimd.memset(t_negh, -0.5)
        t_dummy = const.tile([1, 256], BF16)
        nc.vector.memset(t_dummy, 0.0)
        # ACT warmup: triggers the sigmoid act-table load at t~0
        t_warm = head.tile([128, 1], F32, tag="warm")
        nc.scalar.activation(t_warm, t_negh, AF.Sigmoid)

        # ---- input DMAs (SP engine, ordered by first use) ----
        t_bd = []
        for i, n in enumerate(BD_SPLIT):
            tb = const.tile([CH, n * CH], BF16, tag=f"bd{i}", name=f"bd{i}")
            t_bd.append(tb)
        nc.sync.dma_start(out=t_bd[0], in_=d_bd[0][:])
        t_x = const.tile([CH, NCHUNK, CHS[0]], BF16)
        nc.sync.dma_start(out=t_x, in_=d_x[:])
        t_p1 = const.tile([128, P1_COLS], BF16)
        nc.sync.dma_start(out=t_p1, in_=d_p1[:])
        for i in range(1, len(BD_SPLIT)):
            nc.sync.dma_start(out=t_bd[i], in_=d_bd[i][:])
        t_p2 = const.tile([128, P2_COLS], BF16)
        nc.sync.dma_start(out=t_p2, in_=d_p2[:])
        t_b = []
        for l in range(4):
            if d_b[l] is not None:
                tb = const.tile([1, DIMS[l + 1]], BF16, tag=f"bias{l}",
                                name=f"bias{l}")
                nc.sync.dma_start(out=tb, in_=d_b[l][:])
                t_b.append(tb)
            else:
                t_b.append(None)
        t_convb = None
        if d_convb is not None:
            t_convb = const.tile([1, C_CONV], BF16)
            nc.sync.dma_start(out=t_convb, in_=d_convb[:])
        t_bnscale = t_bnshift = None
        if not bn_uniform:
            t_bnscale = const.tile([T, 1], F32)
            nc.sync.dma_start(out=t_bnscale, in_=d_bnscale[:])
            t_bnshift = const.tile([T, 1], F32)
            nc.sync.dma_start(out=t_bnshift, in_=d_bnshift[:])

        def ahat_chunk(k):
            i = k // 8
            return t_bd[i][:, (k - BD_BASE[i]) * CH:(k - BD_BASE[i] + 1) * CH]

        # block-diagonal W stacks (one mm2 per stacked slot, all bases 0)
        t_wb = {1: t_p1[0:96, P1_W2B:P1_W2B + 96],
                2: t_p1[0:96, P1_W3B:P1_W3B + 192],
                3: t_p1[0:128, P1_W4B:P1_W4B + 304]}
        t_w4s = t_p1[0:64, P1_W4B:P1_W4B + 152]   # plain W4 (block 0)
        t_poolm = t_p1[0:CH, P1_PM:P1_PM + CPG]
        t_ones = t_p2[0:1, P2_ONES:P2_ONES + 128]
        t_wc1 = t_p2[0:128, P2_WC1:P2_WC1 + 816].rearrange("p (k c) -> p k c", k=3)
        t_wc2 = t_p2[0:CHS[3] - 128, P2_WC2:P2_WC2 + 816].rearrange(
            "p (k c) -> p k c", k=3)

        # ---- PE p-state warmup: short dummy chain (ends before inputs land)
        ps_warm = psA.tile([128, 512], F32, tag="m1")
        for _ in range(6):
            nc.tensor.matmul(ps_warm[64:65, 0:256], lhsT=t_dummy[:, 0:1],
                             rhs=t_dummy[:, :], start=True, stop=True)

        # ---- head state ----
        # pooled accumulators share ONE PSUM bank: pooled1 (128,120) at cols
        # [0,120), pooled2 (24,120) at cols [128,248)
        ps_pool = psC.tile([128, 512], F32, tag="pool")
        t_pt1 = [head.tile([128, T + 2], BF16, tag=f"pt1_{b}", name=f"pt1_{b}")
                 for b in range(BPC)]
        t_pt2 = [head.tile([CHS[3] - 128, T + 2], BF16, tag=f"pt2_{b}",
                           name=f"pt2_{b}") for b in range(BPC)]
        t_s = [head.tile([T, C_CONV], F32, tag=f"s{b}", name=f"s{b}")
               for b in range(BPC)]
        t_q = [head.tile([T, C_CONV], F32, tag=f"q{b}", name=f"q{b}")
               for b in range(BPC)]
        t_ssum = head.tile([T, BPC * NCLS], F32, tag="ssum")
        t_y = head.tile([T, BPC * NCLS], F32, tag="y")
        for b in range(BPC):
            for pt in (t_pt1[b], t_pt2[b]):
                nc.gpsimd.memset(pt[:, 0:1], 0.0)
                nc.gpsimd.memset(pt[:, T + 1:T + 2], 0.0)
        caps = {}

        def head_block(b, blk):
            """conv + sigmoid + (s-.5)^2/4 + capsule-reduce for t-block blk of
            clip b. Block 0 runs once 13 of the clip's 24 chunks are pooled;
            block 1 (partition base 64) runs after the clip completes."""
            t0, tn = TBLK[blk]
            # pt col c+1 <- pooled col c; block 0 copies pooled [0,65) (its
            # k=2 tap needs col 64), block 1 copies [65,120); pt cols 0 and
            # 121 are zero edges (memset).
            p0, p1 = (0, t0 + tn + 1) if blk == 0 else (t0 + 1, T)
            sched.copy(t_pt1[b][:, p0 + 1:p1 + 1], ps_pool[:, p0:p1], p1 - p0)
            sched.copy(t_pt2[b][:, p0 + 1:p1 + 1],
                       ps_pool[:CHS[3] - 128, 128 + p0:128 + p1], p1 - p0)
            if b not in caps:
                caps[b] = psB.tile([128, 512], F32, tag="h", name=f"caps{b}")
            ps_caps = caps[b]
            first = True
            for ci, (pt, twc) in enumerate(((t_pt1[b], t_wc1), (t_pt2[b], t_wc2))):
                for kk in range(3):
                    last = (t_convb is None) and ci == 1 and kk == 2
                    nc.tensor.matmul(ps_caps[t0:t0 + tn, :C_CONV],
                                     lhsT=pt[:, t0 + kk:t0 + kk + tn],
                                     rhs=twc[:, kk, :], start=first, stop=last,
                                     skip_group_check=True)
                    first = False
            if t_convb is not None:
                nc.tensor.matmul(ps_caps[t0:t0 + tn, :C_CONV],
                                 lhsT=t_ones[:, :tn], rhs=t_convb[:],
                                 start=False, stop=True, skip_group_check=True)
            sl = slice(t0, t0 + tn)
            if bn_uniform:
                nc.scalar.activation(t_s[b][sl], ps_caps[sl, :C_CONV], AF.Sigmoid)
            else:
                nc.scalar.activation(t_s[b][sl], ps_caps[sl, :C_CONV], AF.Sigmoid,
                                     bias=t_bnshift[sl, 0:1],
                                     scale=t_bnscale[sl, 0:1])
            sched.note("act", C_CONV)
            # q = ((s - .5) * .5)^2  (folds the 4/DIM_CAP=0.25 scale)
            nc.vector.tensor_scalar(
                out=t_q[b][sl], in0=t_s[b][sl], scalar1=-0.5, scalar2=0.5,
                op0=mybir.AluOpType.add, op1=mybir.AluOpType.mult)
            nc.vector.tensor_mul(t_q[b][sl], t_q[b][sl], t_q[b][sl])
            nc.vector.reduce_sum(
                out=t_ssum[sl, b * NCLS:(b + 1) * NCLS],
                in_=t_q[b][sl].rearrange("p (d c) -> p c d", c=NCLS),
                axis=mybir.AxisListType.X)
            sched.note("dve", 3 * C_CONV)

        def head_after_htile(h3, h_base, h_fill):
            """Mean-pool matmuls for a fresh L3 h-tile + head block hooks."""
            for j in range(h_fill):
                k = h_base + j
                kk = k % KPB
                nc.tensor.matmul(ps_pool[:, kk * CPG:(kk + 1) * CPG],
                                 lhsT=h3[:, k, :128], rhs=t_poolm[:],
                                 start=True, stop=True)
                nc.tensor.matmul(
                    ps_pool[:CHS[3] - 128, 128 + kk * CPG:128 + (kk + 1) * CPG],
                    lhsT=h3[:, k, 128:], rhs=t_poolm[:],
                    start=True, stop=True)
            kend = h_base + h_fill
            b, kk = divmod(kend, KPB)
            if kk == 15:            # pooled cols 0..74 ready: t-block 0 (needs 65)
                head_block(b, 0)
            elif kk == 0 and kend > 0:
                head_block(b - 1, 1)

        # ---- L0: W1 folded into xp -> one matmul per chunk. h0 is 32 wide
        # with zero-padded cols [16:32) (one Pool memset) so L1's stacked mm1
        # writes full 32-row blocks.
        h0 = state.tile([CH, NCHUNK, C0P], BF16, tag="h0")
        nc.gpsimd.memset(h0[:, :, CHS[0]:], 0.0)
        for c0 in range(0, NCHUNK, 32):
            ncs = min(32, NCHUNK - c0)
            ps_h = psB.tile([128, 512], F32, tag="h")
            for j in range(ncs):
                k = c0 + j
                nc.tensor.matmul(
                    ps_h[:CH, j * CHS[0]:(j + 1) * CHS[0]],
                    lhsT=ahat_chunk(k), rhs=t_x[:, k, :],
                    start=True, stop=(t_b[0] is None))
                if t_b[0] is not None:
                    nc.tensor.matmul(
                        ps_h[:CH, j * CHS[0]:(j + 1) * CHS[0]],
                        lhsT=t_ones[:, :CH], rhs=t_b[0][:],
                        start=False, stop=True, skip_group_check=True)
                if (j + 1) % 8 == 0:   # evict per 8 chunks (bd wavefront)
                    e0 = j - 7
                    dst = h0[:, c0 + e0:c0 + j + 1, :CHS[0]]
                    src3 = ps_h[:CH, e0 * CHS[0]:(j + 1) * CHS[0]].rearrange(
                        "p (a b) -> p a b", a=8)
                    sched.relu(dst, src3, 8 * CHS[0])
        h_prev = h0

        # ---- L1..L3: software-pipelined mm1 / m1-copy / blocked-mm2 ----
        nlayers = int(os.environ.get("KSTAGE", "4"))
        for l in range(1, nlayers):
            cin, cout = DIMS[l], DIMS[l + 1]
            if l == 1:
                cin = C0P    # zero-padded; W2blk pad rows are zeros
            cpb = H_CPB[cout]
            h_next = state.tile([CH, NCHUNK, cout], BF16, tag=f"h{l}")
            # tiles are 1 PSUM bank (4 slots); L1/L2: 12 chunks (4 slots x 3
            # stacked); L3 alternates pair-tiles (8 chunks, bases {0,64}) and
            # single-tiles (4 chunks, base 0)
            if l < 3:
                tiles = [(c, min(12, NCHUNK - c), "t") for c in range(0, NCHUNK, 12)]
            else:
                tiles = []
                for c in range(0, NCHUNK, 12):
                    tiles.append((c, 12, "p"))   # pair members (8 chunks)
                    tiles.append((c, 12, "s"))   # singles (4 chunks)

            def tile_chunks(t):
                c, n, kind = t
                if kind == "t":
                    return [(c + j, j // 3, j % 3) for j in range(n)]
                if kind == "p":
                    return [(c + tt * 3 + r, tt, r) for tt in range(4)
                            for r in range(2)]
                return [(c + tt * 3 + 2, tt, 0) for tt in range(4)]

            m1_sbs = {}
            ps_h = None
            h_fill = 0
            h_base = 0

            def flush_h(kend):
                nonlocal ps_h, h_fill, h_base
                dst = h_next[:, h_base:h_base + h_fill, :].rearrange(
                    "p a b -> p (a b)")
                sched.relu(dst, ps_h[:CH, :h_fill * cout], h_fill * cout)
                if l == 3:
                    head_after_htile(h_next, h_base, h_fill)
                ps_h = None

            def emit_mm2_slot(m1_sb, sl, kind, k0):
                """One matmul consuming a stacked slot (2-3 chunks)."""
                nonlocal ps_h, h_fill, h_base
                nsk = 3 if kind == "t" else (2 if kind == "p" else 1)
                if ps_h is None:
                    ps_h = psB.tile([128, 512], F32, tag="h")
                    h_fill = 0
                    h_base = k0
                if kind == "t":
                    lhs = m1_sb[0:96, sl * CH:(sl + 1) * CH]
                    rhs = t_wb[l][:, :]
                elif kind == "p":
                    lhs = m1_sb[0:128, sl * CH:(sl + 1) * CH]
                    rhs = t_wb[3][:, :]
                else:
                    lhs = m1_sb[0:64, sl * CH:(sl + 1) * CH]
                    rhs = t_w4s[:, :]
                nc.tensor.matmul(
                    ps_h[:CH, h_fill * cout:(h_fill + nsk) * cout],
                    lhsT=lhs, rhs=rhs, start=True, stop=(t_b[l] is None))
                if t_b[l] is not None:
                    for q in range(nsk):
                        nc.tensor.matmul(
                            ps_h[:CH, (h_fill + q) * cout:(h_fill + q + 1) * cout],
                            lhsT=t_ones[:, :CH], rhs=t_b[l][:],
                            start=False, stop=(q == nsk - 1),
                            skip_group_check=True)
                h_fill += nsk
                if h_fill == cpb or k0 + nsk == NCHUNK:
                    flush_h(k0 + nsk)

            for ti in range(len(tiles) + 1):
                if ti < len(tiles):
                    c, n, kind = tiles[ti]
                    ps_m1 = psA.tile([128, 512], F32, tag="m1")
                    for (k, sl, v) in tile_chunks(tiles[ti]):
                        base = v * 32 if kind == "t" else v * 64
                        nc.tensor.matmul(
                            ps_m1[base:base + cin, sl * CH:(sl + 1) * CH],
                            lhsT=h_prev[:, k, :cin], rhs=ahat_chunk(k),
                            start=True, stop=True)
                    m1_sb = m1p.tile([128, 4 * CH], BF16, tag="m1sb")
                    puse = {"t": 96, "p": 128, "s": 64}[kind]
                    sched.copy(m1_sb[:puse, :4 * CH], ps_m1[:puse, :4 * CH],
                               4 * CH)
                    m1_sbs[ti] = m1_sb
                # mm2 lags the mm1/copy stream by 2 tiles so the in-order PE
                # always has a tile of mm1 work while a copy is in flight.
                if l < 3:
                    todo = []
                    if ti >= 2:
                        todo.append(ti - 2)
                    if ti == len(tiles):
                        todo.append(ti - 1)
                    for t2 in todo:
                        c, n, kind = tiles[t2]
                        for sl in range(4):
                            emit_mm2_slot(m1_sbs[t2], sl, "t", c + sl * 3)
                else:
                    todo = []
                    if ti >= 4 and ti % 2 == 0:
                        todo.append((ti - 4) // 2)
                    if ti == len(tiles):
                        todo.append(len(tiles) // 2 - 1)
                    for g in todo:
                        c = tiles[2 * g][0]
                        for tt in range(4):
                            emit_mm2_slot(m1_sbs[2 * g], tt, "p", c + tt * 3)
                            emit_mm2_slot(m1_sbs[2 * g + 1], tt, "s",
                                          c + tt * 3 + 2)
            h_prev = h_next

        # ---- debug: dump h_prev chunk 0/1 columns instead of the real out ----
        if os.environ.get("KSTAGE"):
            dbg = head.tile([CH, 2 * NCLS], F32, tag="dbg")
            nc.vector.tensor_copy(dbg[:, :NCLS], h_prev[:, 0, :NCLS])
            nc.vector.tensor_copy(dbg[:, NCLS:], h_prev[:, 1, :NCLS])
            nc.sync.dma_start(out=d_out[:CH, :],
                              in_=dbg.rearrange("p (a b) -> p a b", a=2)[:, 0])
            nc.sync.dma_start(out=d_out[CH:2 * CH, :],
                              in_=dbg.rearrange("p (a b) -> p a b", a=2)[:, 1])
            return nc

        # ---- tail: combined sqrt (0.25 scale folded into q) + one DMA ----
        nc.scalar.activation(t_y, t_ssum, AF.Sqrt)
        nc.sync.dma_start(
            out=d_out[:].rearrange("(b t) c -> t b c", b=BPC),
            in_=t_y.rearrange("p (b c) -> p b c", b=BPC))

    return nc


def kernel(x, edge_index, batch, edge_attr, W1, b1, W2, b2, W3, b3, W4, b4,
           conv_w, conv_b, bn_gamma, bn_beta):
    global LAST, LAST_NC
    bd, xp, wct, bn_uniform, cscale = _host_prep(
        x, edge_index, edge_attr, conv_w, W1, bn_gamma, bn_beta)

    bs = [np.asarray(b_, np.float32) for b_ in (b1, b2, b3, b4)]
    nonzero_b = [bool(np.any(b_)) for b_ in bs]
    convb = np.asarray(conv_b, np.float32) * cscale
    nonzero_convb = bool(np.any(convb))

    poolm = np.zeros((CH, CPG), np.float32)
    for j in range(CPG):
        poolm[j * P:(j + 1) * P, j] = 1.0

    pack1 = np.zeros((128, P1_COLS), np.float32)
    for v in range(3):   # W2 block-diag (pad rows stay zero)
        pack1[v * 32:v * 32 + 16, P1_W2B + v * 32:P1_W2B + v * 32 + 32] = \
            np.asarray(W2, np.float32)
    for v in range(3):
        pack1[v * 32:v * 32 + 32, P1_W3B + v * 64:P1_W3B + v * 64 + 64] = \
            np.asarray(W3, np.float32)
    for v in range(2):
        pack1[v * 64:v * 64 + 64, P1_W4B + v * 152:P1_W4B + v * 152 + 152] = \
            np.asarray(W4, np.float32)
    pack1[0:CH, P1_PM:P1_PM + CPG] = poolm

    pack2 = np.zeros((128, P2_COLS), np.float32)
    pack2[0:128, P2_WC1:P2_WC1 + 816] = wct[:128].reshape(128, -1)
    pack2[0:CHS[3] - 128, P2_WC2:P2_WC2 + 816] = wct[128:].reshape(CHS[3] - 128, -1)
    pack2[0:1, P2_ONES:P2_ONES + 128] = 1.0

    nc = _build(nonzero_b, nonzero_convb, bn_uniform, cscale)
    if not nc.is_finalized():
        nc.finalize()   # Bacc: runs the wait-splitting/regalloc compile passes
    LAST_NC = nc

    bnscale = (np.asarray(bn_gamma, np.float32) / np.sqrt(1.0 + BN_EPS)).reshape(T, 1)
    bnshift = np.asarray(bn_beta, np.float32).reshape(T, 1)

    in_maps = []
    for c in range(NCORES):
        m = dict(
            xp=np.ascontiguousarray(xp[c]),
            pack1=pack1.astype(NPBF16),
            pack2=pack2.astype(NPBF16),
        )
        for i, n in enumerate(BD_SPLIT):
            m[f"bd{i}"] = np.ascontiguousarray(
                bd[c][:, BD_BASE[i] * CH:(BD_BASE[i] + n) * CH])
        for l in range(4):
            if nonzero_b[l]:
                m[f"b{l}"] = bs[l].reshape(1, -1).astype(NPBF16)
        if nonzero_convb:
            m["convb"] = convb.reshape(1, -1).astype(NPBF16)
        if not bn_uniform:
            m["bnscale"] = bnscale
            m["bnshift"] = bnshift
        in_maps.append(m)

    LAST = run_bass_kernel_spmd(nc, in_maps, core_ids=list(range(NCORES)),
                                trace=TRACE)
    outs = [LAST.results[c]["out"] for c in range(NCORES)]
    return np.concatenate(outs, axis=0).reshape(BS, T, NCLS)


# revision 36
# speedup vs baseline: 1.1317x; 1.0001x over previous
"""Trainium2 Bass kernel for nn_BaseContextAwareModel (4-layer GCN + mean-pool + conv1d head).

Strategy (per the graph-id sharding hint):
- Each of the 1920 22-node frame-graphs is independent -> shard 240 graphs/core
  across 8 NeuronCores (== 2 batch items per core, since 120 graphs = one clip).
- On host: build the dense normalized adjacency Ahat (G,22,22) from
  edge_index/edge_attr (GCN norm: D^-1/2 (A+I) D^-1/2), then pack 5 graphs per
  128-partition tile as a 110x110 block-diagonal stationary (transposed).
- On device, per 110-node chunk and GCN layer l:
    mm1: M1t = H_c^T AhatT_c     (lhsT = H_c, rhs = AhatT block)   -> PSUM (C_l, 110)
    mm2: H'  = M1t^T W_l (+ b_l) (lhsT = M1t,  rhs = W_l)          -> PSUM (110, C_out)
  with relu fused into the PSUM->SBUF eviction. All matmuls bf16 with f32 PSUM.
- mm1 outputs for 2-3 chunks are stacked at PSUM partition bases {0,32,64}
  (PE-legal tile columns), so each PSUM->SBUF m1 copy moves 2-3 chunks per
  110-column slot: 2-3x less eviction free-size. mm2 consumes a whole stacked
  slot with ONE matmul against a block-diagonal W stack, keeping its tile base
  fixed at 0 (switching the PE row-tile base between matmuls crashes the
  runtime). L3 (cin=64) stacks pairs plus unstacked singles so h-tiles stay at
  3 chunks (456 cols, one PSUM bank; cross-bank compute APs are illegal).
- The PSUM->SBUF traffic (m1 copies + relu evictions) is balanced across the
  Activation and DVE engines by a greedy finish-time scheduler (Pool/GPSIMD
  cannot access PSUM; it does SBUF memsets instead).
- Inputs ship in ~9 large DMAs (HWDGE fixed cost and SP sequencer time
  dominate small DMAs); the block-diag adjacency streams in 8-chunk slices so
  compute starts as soon as the first slice lands.
- PE p-state is warmed with a short chain of dummy matmuls at t=0; the ACT
  sigmoid table is preloaded at t=0 by a warmup op (the only other table,
  Sqrt, is switched to once at the tail, hidden behind DVE work).
- Head: mean-pool via 0/1 matmul into a per-clip PSUM accumulator bank (1/22
  and the uniform eval-BN scale folded into conv weights on host), conv1d(k=3)
  as shifted-lhsT matmuls split into t-blocks [0,64) and [64,120) so most of
  the chain runs before the last chunks finish, sigmoid on ACT, (s-.5)^2/4 and
  the capsule reduction on DVE, one combined Sqrt, one output DMA.
"""

import os
from contextlib import ExitStack

import numpy as np

import concourse.bass as bass
import concourse.bacc as bacc
import concourse.tile as tile
from concourse import mybir
from concourse.bass_utils import run_bass_kernel_spmd

# ---- problem constants (hardcoded; kernel.py must be self-contained) ----
BS, T, P, G = 16, 120, 22, 1920
NCORES = 8
GPC = G // NCORES          # 240 graphs per core
CPG = 5                    # graphs per 128-partition chunk
CH = CPG * P               # 110 nodes per chunk
NCHUNK = GPC // CPG        # 48 chunks per core
BPC = BS // NCORES         # 2 batch items (clips) per core
KPB = T // CPG             # 24 chunks per clip
C_IN = 14
CHS = [16, 32, 64, 152]
DIMS = [C_IN] + CHS
NCLS, DIM_CAP = 17, 16
C_CONV = DIM_CAP * NCLS    # 272
BN_EPS = 1e-3

C0P = 32                   # L0 output channels padded 16->32 (device memset)
H_CPB = {32: 12, 64: 6, 152: 3}   # chunks per 1-bank h supertile
# conv/head t-blocks: (t0, tn); second block at partition base 64 (PE-legal)
TBLK = [(0, 64), (64, 56)]

F32 = mybir.dt.float32
BF16 = mybir.dt.bfloat16
NPBF16 = np.dtype(mybir.dt.np(BF16))

TRACE = os.environ.get("KTRACE", "0") == "1"
LAST = None  # last BassKernelResults, for test harness introspection
LAST_NC = None  # last built bass.Bass module, for cost-model simulation

# const-pack column layout (bf16):
# pack1: W2blk (96,96) | W3blk (96,192) | W4blk (128,304) | poolm (110,5)
P1_W2B, P1_W3B, P1_W4B, P1_PM = 0, 96, 288, 592
P1_COLS = 597
# pack2: wc1 [0:128,0:816] | wc2 [0:24,816:1632] | ones [0:1,1632:1760]
P2_WC1, P2_WC2, P2_ONES = 0, 816, 1632
P2_COLS = 1760

# bd DMA split (chunks per slice)
BD_SPLIT = [8, 8, 8, 8, 8, 8]
BD_BASE = list(np.cumsum([0] + BD_SPLIT))


def _host_prep(x, edge_index, edge_attr, conv_w, W1, bn_gamma, bn_beta):
    """Dense normalized adjacency + per-core packed operands."""
    src = np.asarray(edge_index[0], np.int64)
    dst = np.asarray(edge_index[1], np.int64)
    w = np.asarray(edge_attr[:, 4], np.float32)

    A = np.zeros((G, P, P), np.float32)
    np.add.at(A, (dst // P, dst % P, src % P), w)
    deg = A.sum(axis=2) + 1.0                      # + self-loop weight 1
    dinv = 1.0 / np.sqrt(deg)                      # deg >= 1 always
    Ahat = dinv[:, :, None] * A * dinv[:, None, :]
    ii = np.arange(P)
    Ahat[:, ii, ii] += dinv * dinv                 # self loop: dinv[d]^2
    AhatT = np.ascontiguousarray(Ahat.transpose(0, 2, 1))  # [g, s, d]

    # block-diag pack: (NCORES, CH, NCHUNK*CH); rows = source node in chunk,
    # cols = chunk*CH + dest node in chunk
    bd = np.zeros((NCORES, CH, NCHUNK * CH), np.float32)
    bdv = bd.reshape(NCORES, CH, NCHUNK, CH)
    Ar = AhatT.reshape(NCORES, NCHUNK, CPG, P, P)
    for j in range(CPG):
        bdv[:, j * P:(j + 1) * P, :, j * P:(j + 1) * P] = \
            Ar[:, :, j].transpose(0, 2, 1, 3)

    # layer-1 W folded on host: ship XW1 = x @ W1, packed (8, 110, 48, 16)
    xw = np.asarray(x, np.float32) @ np.asarray(W1, np.float32)
    xr = xw.reshape(NCORES, NCHUNK, CH, CHS[0])
    xp = np.ascontiguousarray(xr.transpose(0, 2, 1, 3))

    # conv weights: (co, ci, k) -> (ci, k, co), with the 1/22 mean-pool factor.
    # BatchNorm(eval) with uniform gamma and zero beta folds into the conv
    # weights as a scalar (the conv bias is scaled on the host side too).
    gamma = np.asarray(bn_gamma, np.float32)
    beta = np.asarray(bn_beta, np.float32)
    bn_uniform = bool(np.allclose(gamma, gamma[0]) and not np.any(beta))
    cscale = float(gamma[0] / np.sqrt(1.0 + BN_EPS)) if bn_uniform else 1.0
    wct = np.asarray(conv_w, np.float32).transpose(1, 2, 0) * (cscale / float(P))
    return bd.astype(NPBF16), xp.astype(NPBF16), np.ascontiguousarray(wct), \
        bn_uniform, cscale


class EvictSched:
    """Greedy finish-time balancing of PSUM->SBUF copies/activations across
    the ACT and DVE engines (Pool/GPSIMD cannot access PSUM)."""

    def __init__(self, nc):
        self.nc = nc
        self.est = {"act": 0.0, "dve": 0.0}

    def _cost(self, eng, cols):
        if eng == "act":
            return cols * 0.833 + 185.0
        return cols * 1.042 + 125.0

    def _pick(self, cols):
        eng = min(self.est, key=lambda e: self.est[e] + self._cost(e, cols))
        self.est[eng] += self._cost(eng, cols)
        return eng

    def note(self, eng, cols):
        self.est[eng] += self._cost(eng, cols)

    def copy(self, dst, src, cols, pin=None):
        eng = pin or self._pick(cols)
        if pin:
            self.note(eng, cols)
        nc = self.nc
        if eng == "act":
            nc.scalar.activation(dst, src, mybir.ActivationFunctionType.Copy)
        else:
            nc.vector.tensor_copy(dst, src)
        return eng

    def relu(self, dst, src, cols, pin=None):
        eng = pin or self._pick(cols)
        if pin:
            self.note(eng, cols)
        nc = self.nc
        if eng == "act":
            nc.scalar.activation(dst, src, mybir.ActivationFunctionType.Relu)
        else:
            nc.vector.tensor_scalar_max(dst, src, 0.0)
        return eng


def _build(nonzero_b, nonzero_convb, bn_uniform, cscale):
    """Build the SPMD Bass program (identical on all 8 cores)."""
    nc = bacc.Bacc()
    AF = mybir.ActivationFunctionType

    d_bd = [nc.declare_dram_parameter(f"bd{i}", [CH, n * CH], BF16, isOutput=False)
            for i, n in enumerate(BD_SPLIT)]
    d_x = nc.declare_dram_parameter("xp", [CH, NCHUNK, CHS[0]], BF16, isOutput=False)
    d_p1 = nc.declare_dram_parameter("pack1", [128, P1_COLS], BF16, isOutput=False)
    d_p2 = nc.declare_dram_parameter("pack2", [128, P2_COLS], BF16, isOutput=False)
    d_b = [nc.declare_dram_parameter(f"b{l}", [1, DIMS[l + 1]], BF16, isOutput=False)
           if nonzero_b[l] else None for l in range(4)]
    d_convb = (nc.declare_dram_parameter("convb", [1, C_CONV], BF16, isOutput=False)
               if nonzero_convb else None)
    if not bn_uniform:
        d_bnscale = nc.declare_dram_parameter("bnscale", [T, 1], F32, isOutput=False)
        d_bnshift = nc.declare_dram_parameter("bnshift", [T, 1], F32, isOutput=False)
    d_out = nc.declare_dram_parameter("out", [BPC * T, NCLS], F32, isOutput=True)

    with tile.TileContext(nc) as tc, ExitStack() as ctx:
        const = ctx.enter_context(tc.tile_pool(name="const", bufs=1))
        state = ctx.enter_context(tc.tile_pool(name="state", bufs=1))
        m1p = ctx.enter_context(tc.tile_pool(name="m1sb", bufs=6))
        psA = ctx.enter_context(tc.tile_pool(name="psA", bufs=3, space="PSUM"))
        psB = ctx.enter_context(tc.tile_pool(name="psB", bufs=2, space="PSUM"))
        psC = ctx.enter_context(tc.tile_pool(name="psC", bufs=1, space="PSUM"))
        head = ctx.enter_context(tc.tile_pool(name="head", bufs=1))

        sched = EvictSched(nc)

        # ---- warmups (no DMA dependencies) ----
        t_negh = const.tile([128, 1], F32)
        nc.gpsimd.memset(t_negh, -0.5)
        t_dummy = const.tile([1, 256], BF16)
        nc.vector.memset(t_dummy, 0.0)
        # ACT warmup: triggers the sigmoid act-table load at t~0
        t_warm = head.tile([128, 1], F32, tag="warm")
        nc.scalar.activation(t_warm, t_negh, AF.Sigmoid)
        t_half = const.tile([128, 1], F32)
        nc.gpsimd.memset(t_half, 0.5)
        t_negq = const.tile([128, 1], F32)
        nc.gpsimd.memset(t_negq, -0.25)

        # ---- input DMAs (SP engine, ordered by first use) ----
        t_bd = []
        for i, n in enumerate(BD_SPLIT):
            tb = const.tile([CH, n * CH], BF16, tag=f"bd{i}", name=f"bd{i}")
            t_bd.append(tb)
        nc.sync.dma_start(out=t_bd[0], in_=d_bd[0][:])
        t_x = const.tile([CH, NCHUNK, CHS[0]], BF16)
        nc.sync.dma_start(out=t_x, in_=d_x[:])
        t_p1 = const.tile([128, P1_COLS], BF16)
        nc.sync.dma_start(out=t_p1, in_=d_p1[:])
        for i in range(1, len(BD_SPLIT)):
            nc.sync.dma_start(out=t_bd[i], in_=d_bd[i][:])
        t_p2 = const.tile([128, P2_COLS], BF16)
        nc.sync.dma_start(out=t_p2, in_=d_p2[:])
        t_b = []
        for l in range(4):
            if d_b[l] is not None:
                tb = const.tile([1, DIMS[l + 1]], BF16, tag=f"bias{l}",
                                name=f"bias{l}")
                nc.sync.dma_start(out=tb, in_=d_b[l][:])
                t_b.append(tb)
            else:
                t_b.append(None)
        t_convb = None
        if d_convb is not None:
            t_convb = const.tile([1, C_CONV], BF16)
            nc.sync.dma_start(out=t_convb, in_=d_convb[:])
        t_bnscale = t_bnshift = None
        if not bn_uniform:
            t_bnscale = const.tile([T, 1], F32)
            nc.sync.dma_start(out=t_bnscale, in_=d_bnscale[:])
            t_bnshift = const.tile([T, 1], F32)
            nc.sync.dma_start(out=t_bnshift, in_=d_bnshift[:])

        def ahat_chunk(k):
            i = k // 8
            return t_bd[i][:, (k - BD_BASE[i]) * CH:(k - BD_BASE[i] + 1) * CH]

        # block-diagonal W stacks (one mm2 per stacked slot, all bases 0)
        t_wb = {1: t_p1[0:96, P1_W2B:P1_W2B + 96],
                2: t_p1[0:96, P1_W3B:P1_W3B + 192],
                3: t_p1[0:128, P1_W4B:P1_W4B + 304]}
        t_w4s = t_p1[0:64, P1_W4B:P1_W4B + 152]   # plain W4 (block 0)
        t_poolm = t_p1[0:CH, P1_PM:P1_PM + CPG]
        t_ones = t_p2[0:1, P2_ONES:P2_ONES + 128]
        t_wc1 = t_p2[0:128, P2_WC1:P2_WC1 + 816].rearrange("p (k c) -> p k c", k=3)
        t_wc2 = t_p2[0:CHS[3] - 128, P2_WC2:P2_WC2 + 816].rearrange(
            "p (k c) -> p k c", k=3)

        # ---- PE p-state warmup: short dummy chain (ends before inputs land)
        ps_warm = psA.tile([128, 512], F32, tag="m1")
        for _ in range(6):
            nc.tensor.matmul(ps_warm[64:65, 0:256], lhsT=t_dummy[:, 0:1],
                             rhs=t_dummy[:, :], start=True, stop=True)

        # ---- head state ----
        # pooled accumulators share ONE PSUM bank: pooled1 (128,120) at cols
        # [0,120), pooled2 (24,120) at cols [128,248)
        ps_pool = psC.tile([128, 512], F32, tag="pool")
        t_pt1 = [head.tile([128, T + 2], BF16, tag=f"pt1_{b}", name=f"pt1_{b}")
                 for b in range(BPC)]
        t_pt2 = [head.tile([CHS[3] - 128, T + 2], BF16, tag=f"pt2_{b}",
                           name=f"pt2_{b}") for b in range(BPC)]
        t_s = [head.tile([T, C_CONV], F32, tag=f"s{b}", name=f"s{b}")
               for b in range(BPC)]
        t_q = [head.tile([T, C_CONV], F32, tag=f"q{b}", name=f"q{b}")
               for b in range(BPC)]
        t_ssum = head.tile([T, BPC * NCLS], F32, tag="ssum")
        t_y = head.tile([T, BPC * NCLS], F32, tag="y")
        for b in range(BPC):
            for pt in (t_pt1[b], t_pt2[b]):
                nc.gpsimd.memset(pt[:, 0:1], 0.0)
                nc.gpsimd.memset(pt[:, T + 1:T + 2], 0.0)
        caps = {}

        def head_block(b, blk):
            """conv + sigmoid + (s-.5)^2/4 + capsule-reduce for t-block blk of
            clip b. Block 0 runs once 13 of the clip's 24 chunks are pooled;
            block 1 (partition base 64) runs after the clip completes."""
            t0, tn = TBLK[blk]
            # pt col c+1 <- pooled col c; block 0 copies pooled [0,65) (its
            # k=2 tap needs col 64), block 1 copies [65,120); pt cols 0 and
            # 121 are zero edges (memset).
            p0, p1 = (0, t0 + tn + 1) if blk == 0 else (t0 + 1, T)
            sched.copy(t_pt1[b][:, p0 + 1:p1 + 1], ps_pool[:, p0:p1], p1 - p0)
            sched.copy(t_pt2[b][:, p0 + 1:p1 + 1],
                       ps_pool[:CHS[3] - 128, 128 + p0:128 + p1], p1 - p0)
            if b not in caps:
                caps[b] = psB.tile([128, 512], F32, tag="h", name=f"caps{b}")
            ps_caps = caps[b]
            first = True
            for ci, (pt, twc) in enumerate(((t_pt1[b], t_wc1), (t_pt2[b], t_wc2))):
                for kk in range(3):
                    last = (t_convb is None) and ci == 1 and kk == 2
                    nc.tensor.matmul(ps_caps[t0:t0 + tn, :C_CONV],
                                     lhsT=pt[:, t0 + kk:t0 + kk + tn],
                                     rhs=twc[:, kk, :], start=first, stop=last,
                                     skip_group_check=True)
                    first = False
            if t_convb is not None:
                nc.tensor.matmul(ps_caps[t0:t0 + tn, :C_CONV],
                                 lhsT=t_ones[:, :tn], rhs=t_convb[:],
                                 start=False, stop=True, skip_group_check=True)
            sl = slice(t0, t0 + tn)
            if bn_uniform:
                nc.scalar.activation(t_s[b][sl], ps_caps[sl, :C_CONV], AF.Sigmoid)
            else:
                nc.scalar.activation(t_s[b][sl], ps_caps[sl, :C_CONV], AF.Sigmoid,
                                     bias=t_bnshift[sl, 0:1],
                                     scale=t_bnscale[sl, 0:1])
            sched.note("act", C_CONV)
            # q = ((s - .5) * .5)^2  (folds the 4/DIM_CAP=0.25 scale). For the
            # final block use one ACT Square (same table set) to shorten the
            # tail chain; earlier blocks use DVE to keep ACT free for evicts.
            if (b, blk) == (BPC - 1, 1):
                nc.scalar.activation(t_q[b][sl], t_s[b][sl],
                                     mybir.ActivationFunctionType.Square,
                                     bias=t_negq[sl, 0:1], scale=0.5)
                sched.note("act", C_CONV)
            else:
                nc.vector.tensor_scalar(
                    out=t_q[b][sl], in0=t_s[b][sl], scalar1=-0.5, scalar2=0.5,
                    op0=mybir.AluOpType.add, op1=mybir.AluOpType.mult)
                nc.vector.tensor_mul(t_q[b][sl], t_q[b][sl], t_q[b][sl])
            nc.vector.reduce_sum(
                out=t_ssum[sl, b * NCLS:(b + 1) * NCLS],
                in_=t_q[b][sl].rearrange("p (d c) -> p c d", c=NCLS),
                axis=mybir.AxisListType.X)
            sched.note("dve", 3 * C_CONV)

        def head_after_htile(h3, h_base, h_fill):
            """Mean-pool matmuls for a fresh L3 h-tile + head block hooks."""
            for j in range(h_fill):
                k = h_base + j
                kk = k % KPB
                nc.tensor.matmul(ps_pool[:, kk * CPG:(kk + 1) * CPG],
                                 lhsT=h3[:, k, :128], rhs=t_poolm[:],
                                 start=True, stop=True)
                nc.tensor.matmul(
                    ps_pool[:CHS[3] - 128, 128 + kk * CPG:128 + (kk + 1) * CPG],
                    lhsT=h3[:, k, 128:], rhs=t_poolm[:],
                    start=True, stop=True)
            kend = h_base + h_fill
            b, kk = divmod(kend, KPB)
            if kk == 15:            # pooled cols 0..74 ready: t-block 0 (needs 65)
                head_block(b, 0)
            elif kk == 0 and kend > 0:
                head_block(b - 1, 1)

        # ---- L0: W1 folded into xp -> one matmul per chunk. h0 is 32 wide
        # with zero-padded cols [16:32) (one Pool memset) so L1's stacked mm1
        # writes full 32-row blocks.
        h0 = state.tile([CH, NCHUNK, C0P], BF16, tag="h0")
        nc.gpsimd.memset(h0[:, :, CHS[0]:], 0.0)
        for c0 in range(0, NCHUNK, 32):
            ncs = min(32, NCHUNK - c0)
            ps_h = psB.tile([128, 512], F32, tag="h")
            for j in range(ncs):
                k = c0 + j
                nc.tensor.matmul(
                    ps_h[:CH, j * CHS[0]:(j + 1) * CHS[0]],
                    lhsT=ahat_chunk(k), rhs=t_x[:, k, :],
                    start=True, stop=(t_b[0] is None))
                if t_b[0] is not None:
                    nc.tensor.matmul(
                        ps_h[:CH, j * CHS[0]:(j + 1) * CHS[0]],
                        lhsT=t_ones[:, :CH], rhs=t_b[0][:],
                        start=False, stop=True, skip_group_check=True)
                if (j + 1) % 8 == 0:   # evict per 8 chunks (bd wavefront)
                    e0 = j - 7
                    dst = h0[:, c0 + e0:c0 + j + 1, :CHS[0]]
                    src3 = ps_h[:CH, e0 * CHS[0]:(j + 1) * CHS[0]].rearrange(
                        "p (a b) -> p a b", a=8)
                    sched.relu(dst, src3, 8 * CHS[0])
        h_prev = h0

        # ---- L1..L3: software-pipelined mm1 / m1-copy / blocked-mm2 ----
        nlayers = int(os.environ.get("KSTAGE", "4"))
        for l in range(1, nlayers):
            cin, cout = DIMS[l], DIMS[l + 1]
            if l == 1:
                cin = C0P    # zero-padded; W2blk pad rows are zeros
            cpb = H_CPB[cout]
            h_next = state.tile([CH, NCHUNK, cout], BF16, tag=f"h{l}")
            # tiles are 1 PSUM bank (4 slots); L1/L2: 12 chunks (4 slots x 3
            # stacked); L3 alternates pair-tiles (8 chunks, bases {0,64}) and
            # single-tiles (4 chunks, base 0)
            if l < 3:
                tiles = [(c, min(12, NCHUNK - c), "t") for c in range(0, NCHUNK, 12)]
            else:
                tiles = []
                for c in range(0, NCHUNK, 12):
                    tiles.append((c, 12, "p"))   # pair members (8 chunks)
                    tiles.append((c, 12, "s"))   # singles (4 chunks)

            def tile_chunks(t):
                c, n, kind = t
                if kind == "t":
                    return [(c + j, j // 3, j % 3) for j in range(n)]
                if kind == "p":
                    return [(c + tt * 3 + r, tt, r) for tt in range(4)
                            for r in range(2)]
                return [(c + tt * 3 + 2, tt, 0) for tt in range(4)]

            m1_sbs = {}
            ps_h = None
            h_fill = 0
            h_base = 0

            def flush_bank(bank, nch):
                """Evict one full PSUM bank (cpb chunks) of the h tile."""
                b0 = h_base + bank * cpb
                dst = h_next[:, b0:b0 + nch, :].rearrange("p a b -> p (a b)")
                sched.relu(dst, ps_h[:CH, bank * 512:bank * 512 + nch * cout],
                           nch * cout)
                if l == 3:
                    head_after_htile(h_next, b0, nch)

            def emit_mm2_slot(m1_sb, sl, kind, k0):
                """One matmul consuming a stacked slot (2-3 chunks)."""
                nonlocal ps_h, h_fill, h_base
                nsk = 3 if kind == "t" else (2 if kind == "p" else 1)
                if ps_h is None:
                    ps_h = psB.tile([128, 1024], F32, tag="h")
                    h_fill = 0
                    h_base = k0
                bank, bo = divmod(h_fill, cpb)
                if kind == "t":
                    lhs = m1_sb[0:96, sl * CH:(sl + 1) * CH]
                    rhs = t_wb[l][:, :]
                elif kind == "p":
                    lhs = m1_sb[0:128, sl * CH:(sl + 1) * CH]
                    rhs = t_wb[3][:, :]
                else:
                    lhs = m1_sb[0:64, sl * CH:(sl + 1) * CH]
                    rhs = t_w4s[:, :]
                off = bank * 512 + bo * cout
                nc.tensor.matmul(
                    ps_h[:CH, off:off + nsk * cout],
                    lhsT=lhs, rhs=rhs, start=True, stop=(t_b[l] is None))
                if t_b[l] is not None:
                    for q in range(nsk):
                        nc.tensor.matmul(
                            ps_h[:CH, off + q * cout:off + (q + 1) * cout],
                            lhsT=t_ones[:, :CH], rhs=t_b[l][:],
                            start=False, stop=(q == nsk - 1),
                            skip_group_check=True)
                h_fill += nsk
                if h_fill % cpb == 0 or k0 + nsk == NCHUNK:
                    flush_bank((h_fill - 1) // cpb, h_fill - ((h_fill - 1) // cpb) * cpb)
                    if h_fill == 2 * cpb or k0 + nsk == NCHUNK:
                        ps_h = None

            for ti in range(len(tiles) + 1):
                if ti < len(tiles):
                    c, n, kind = tiles[ti]
                    ps_m1 = psA.tile([128, 512], F32, tag="m1")
                    for (k, sl, v) in tile_chunks(tiles[ti]):
                        base = v * 32 if kind == "t" else v * 64
                        nc.tensor.matmul(
                            ps_m1[base:base + cin, sl * CH:(sl + 1) * CH],
                            lhsT=h_prev[:, k, :cin], rhs=ahat_chunk(k),
                            start=True, stop=True)
                    m1_sb = m1p.tile([128, 4 * CH], BF16, tag="m1sb")
                    puse = {"t": 96, "p": 128, "s": 64}[kind]
                    sched.copy(m1_sb[:puse, :4 * CH], ps_m1[:puse, :4 * CH],
                               4 * CH)
                    m1_sbs[ti] = m1_sb
                # mm2 lags the mm1/copy stream by one tile (group for L3)
                if ti > 0 and l < 3:
                    c, n, kind = tiles[ti - 1]
                    for sl in range(4):
                        emit_mm2_slot(m1_sbs[ti - 1], sl, "t", c + sl * 3)
                elif l == 3 and ((ti >= 3 and ti % 2 == 1) or ti == len(tiles)):
                    g = (ti - 3) // 2 if ti < len(tiles) else len(tiles) // 2 - 1
                    c = tiles[2 * g][0]
                    for tt in range(4):
                        emit_mm2_slot(m1_sbs[2 * g], tt, "p", c + tt * 3)
                        emit_mm2_slot(m1_sbs[2 * g + 1], tt, "s", c + tt * 3 + 2)
            h_prev = h_next

        # ---- debug: dump h_prev chunk 0/1 columns instead of the real out ----
        if os.environ.get("KSTAGE"):
            dbg = head.tile([CH, 2 * NCLS], F32, tag="dbg")
            nc.vector.tensor_copy(dbg[:, :NCLS], h_prev[:, 0, :NCLS])
            nc.vector.tensor_copy(dbg[:, NCLS:], h_prev[:, 1, :NCLS])
            nc.sync.dma_start(out=d_out[:CH, :],
                              in_=dbg.rearrange("p (a b) -> p a b", a=2)[:, 0])
            nc.sync.dma_start(out=d_out[CH:2 * CH, :],
                              in_=dbg.rearrange("p (a b) -> p a b", a=2)[:, 1])
            return nc

        # ---- tail: combined sqrt via GPSIMD pow(x, 0.5) (avoids switching
        # the ACT activation table away from the sigmoid set) + one DMA ----
        nc.gpsimd.tensor_tensor(
            out=t_y, in0=t_ssum,
            in1=t_half[0:T, 0:1].to_broadcast((T, BPC * NCLS)),
            op=mybir.AluOpType.pow)
        nc.sync.dma_start(
            out=d_out[:].rearrange("(b t) c -> t b c", b=BPC),
            in_=t_y.rearrange("p (b c) -> p b c", b=BPC))

    return nc


def kernel(x, edge_index, batch, edge_attr, W1, b1, W2, b2, W3, b3, W4, b4,
           conv_w, conv_b, bn_gamma, bn_beta):
    global LAST, LAST_NC
    bd, xp, wct, bn_uniform, cscale = _host_prep(
        x, edge_index, edge_attr, conv_w, W1, bn_gamma, bn_beta)

    bs = [np.asarray(b_, np.float32) for b_ in (b1, b2, b3, b4)]
    nonzero_b = [bool(np.any(b_)) for b_ in bs]
    convb = np.asarray(conv_b, np.float32) * cscale
    nonzero_convb = bool(np.any(convb))

    poolm = np.zeros((CH, CPG), np.float32)
    for j in range(CPG):
        poolm[j * P:(j + 1) * P, j] = 1.0

    pack1 = np.zeros((128, P1_COLS), np.float32)
    for v in range(3):   # W2 block-diag (pad rows stay zero)
        pack1[v * 32:v * 32 + 16, P1_W2B + v * 32:P1_W2B + v * 32 + 32] = \
            np.asarray(W2, np.float32)
    for v in range(3):
        pack1[v * 32:v * 32 + 32, P1_W3B + v * 64:P1_W3B + v * 64 + 64] = \
            np.asarray(W3, np.float32)
    for v in range(2):
        pack1[v * 64:v * 64 + 64, P1_W4B + v * 152:P1_W4B + v * 152 + 152] = \
            np.asarray(W4, np.float32)
    pack1[0:CH, P1_PM:P1_PM + CPG] = poolm

    pack2 = np.zeros((128, P2_COLS), np.float32)
    pack2[0:128, P2_WC1:P2_WC1 + 816] = wct[:128].reshape(128, -1)
    pack2[0:CHS[3] - 128, P2_WC2:P2_WC2 + 816] = wct[128:].reshape(CHS[3] - 128, -1)
    pack2[0:1, P2_ONES:P2_ONES + 128] = 1.0

    nc = _build(nonzero_b, nonzero_convb, bn_uniform, cscale)
    if not nc.is_finalized():
        nc.finalize()   # Bacc: runs the wait-splitting/regalloc compile passes
    LAST_NC = nc

    bnscale = (np.asarray(bn_gamma, np.float32) / np.sqrt(1.0 + BN_EPS)).reshape(T, 1)
    bnshift = np.asarray(bn_beta, np.float32).reshape(T, 1)

    in_maps = []
    for c in range(NCORES):
        m = dict(
            xp=np.ascontiguousarray(xp[c]),
            pack1=pack1.astype(NPBF16),
            pack2=pack2.astype(NPBF16),
        )
        for i, n in enumerate(BD_SPLIT):
            m[f"bd{i}"] = np.ascontiguousarray(
                bd[c][:, BD_BASE[i] * CH:(BD_BASE[i] + n) * CH])
        for l in range(4):
            if nonzero_b[l]:
                m[f"b{l}"] = bs[l].reshape(1, -1).astype(NPBF16)
        if nonzero_convb:
            m["convb"] = convb.reshape(1, -1).astype(NPBF16)
        if not bn_uniform:
            m["bnscale"] = bnscale
            m["bnshift"] = bnshift
        in_maps.append(m)

    LAST = run_bass_kernel_spmd(nc, in_maps, core_ids=list(range(NCORES)),
                                trace=TRACE)
    outs = [LAST.results[c]["out"] for c in range(NCORES)]
    return np.concatenate(outs, axis=0).reshape(BS, T, NCLS)
